# revision 21
# baseline (speedup 1.0000x reference)
"""Trainium2 Bass kernel for nn_Attention_8143257993917.

Multi-head attention (packed QKV + RoPE + additive bias + softmax + head_mask
+ o_proj), B=4, S=2048, D=1024, H=16 heads, fp32 I/O.

Sharding: 8 cores = 4 batches x 2 head-groups (tensor-parallel over heads).
Core c handles batch b = c // 2 and heads g*8..g*8+8 with g = c % 2.
Each core computes a partial output (its heads' contribution through o_proj);
the host sums the two partials per batch and adds o_b.

Device-side design (per core, fast mode):
- Everything runs in "transposed" feature-major layouts so the big score /
  probability matrices never need an on-chip transpose:
    Q_T, K_T: [f, t] (f = head*64+d on partitions): out[f,t] = wT[d,f].T @ hT.
    RoPE: q' = (q + bq) * cos + (rot(q) + rot(bq)) * sin, where the rotated
      branch comes from a SECOND projection with host-prerotated weights
      (rotate_half is a row permutation+sign of W, so it folds into weights).
    V: [t, f] natural layout, so V chunks [k=128, d=64] are directly the
      stationary operand of the PV matmul. A ones-column appended to V makes
      the PV matmul also produce the softmax denominators (row 64 of ctx).
    scores S_T[k, q] = K_T_chunk.T @ Q_T (contraction d=64), fp16 operands,
      fp32 PSUM accumulate.
    bias: exp(S+b) = exp(S)*exp(b); exp(bias) is precomputed on the host in
      fp16 and applied as one elementwise multiply on VectorE (removes 512
      identity-matmul bias adds from the PE).
    exp on ScalarE (PSUM -> SBUF) with a constant -12 shift (softmax is
      shift-invariant; keeps exp outputs inside fp16 range).
    PV is software-pipelined one k-chunk behind scores/exp/mult so the PE
      never waits on the current chunk's ScalarE/VectorE results.
    softmax denominators: exact VectorE reciprocal on a [32, NQH/32] reshape
      (via a small DRAM round-trip on the gpsimd DMA queues, which also
      broadcasts 1/r across 64 partitions); one TT multiply normalizes ctx
      and moves it PSUM -> SBUF.
    head_mask is folded into the V projection weights/bias on the host.
    o_proj: out_T[o, t] = sum_f o_wT[f, o] * ctx_T[f, t], fp16 operands.
  Matmul dtype is fp16 rather than bf16: same PE throughput, ~8x lower
  quantization error (all value ranges verified to fit fp16 comfortably).
  fp32 matmuls on TRN2 lower to LOW_HIGH double-pass + 2 cycles/column
  streaming (~5x slower than fp16), hence the fp16 datapath with fp32
  accumulation; measured end-to-end relative error vs the fp32 reference
  is ~1.3e-3.
"""

import sys

sys.path.insert(0, "/opt/trn_rl_repo")

import numpy as np

_CACHE = {}

H = 16
HPC = 8  # heads per core
G = 2  # head groups


def build_nc(S=2048, D=1024, fast=True):
    """Build + compile the per-core Bass program (same program on all cores)."""
    import concourse.bass as bass
    from concourse import bacc
    import concourse.mybir as mybir
    import concourse.tile as tile
    from concourse.masks import make_identity
    from concourse.tile_rust import add_dep_helper

    F32 = mybir.dt.float32
    BF16 = mybir.dt.bfloat16
    F16 = mybir.dt.float16
    I32 = mybir.dt.int32
    MT = F16 if fast else F32      # matmul operand dtype
    AF = mybir.ActivationFunctionType

    P = 128
    DC = D // P          # d chunks (contraction for projections)
    KC = S // P          # k chunks (scores contraction)
    NQH = S // 2         # q-half size
    NQ = min(512, NQH)   # matmul free-dim chunk
    NQC = NQH // NQ      # chunks per q-half
    FPC = HPC * 64       # features per core (= 512)
    FT = FPC // P        # f-tiles per tensor (= 4)
    NT = min(512, S)     # phase C t-chunk
    TT4 = S // NT
    NTA = min(512, NQH)  # phase A t-chunk

    nc = bacc.Bacc("TRN2", target_bir_lowering=False, debug=False, num_devices=8)

    hT = nc.dram_tensor("hT", [D, S], MT, kind="ExternalInput")
    w4 = nc.dram_tensor("w4", [D, 2 * FPC], MT, kind="ExternalInput")
    b4 = nc.dram_tensor("b4", [4 * FPC], F32, kind="ExternalInput")
    prot = nc.dram_tensor("prot", [P, P], MT, kind="ExternalInput")
    wvT = nc.dram_tensor("wvT", [D, FPC], MT, kind="ExternalInput")
    bv = nc.dram_tensor("bv", [FPC], MT, kind="ExternalInput")
    cosr = nc.dram_tensor("cosr", [P, S], F32, kind="ExternalInput")
    sinr = nc.dram_tensor("sinr", [P, S], F32, kind="ExternalInput")
    if fast:
        expbT = nc.dram_tensor("expbT", [S, S], F16, kind="ExternalInput")
    else:
        biasT = nc.dram_tensor("biasT", [S, S], F32, kind="ExternalInput")
    owT = nc.dram_tensor("owT", [FPC, D], MT, kind="ExternalInput")
    outT = nc.dram_tensor("outT", [D, S], F32, kind="ExternalOutput")

    hT_r = hT.ap().rearrange("(o p) t -> p o t", p=P)
    w4_r = w4.ap().rearrange("(o p) f -> p o f", p=P)
    wv_r = wvT.ap().rearrange("(o p) f -> p o f", p=P)
    ow_r = owT.ap().rearrange("(o p) f -> p o f", p=P)
    b4_r = b4.ap().rearrange("(o p) -> p o", p=P)

    with tile.TileContext(nc) as tc:
        with (
            tc.tile_pool(name="cst", bufs=1) as cst,
            tc.tile_pool(name="pAB", bufs=1) as pAB,
            tc.tile_pool(name="dram", bufs=4, space="DRAM") as dpool,
        ):
            ident = cst.tile([P, P], F32)
            make_identity(nc, ident)
            prot_sb = cst.tile([P, P], MT)
            nc.sync.dma_start(prot_sb[:], prot.ap())
            ones1 = cst.tile([1, P], MT)
            nc.vector.memset(ones1[:], 1.0)
            b4_sb = cst.tile([P, 4 * FPC // P], F32)
            nc.sync.dma_start(b4_sb[:], b4_r)
            bv_sb = cst.tile([1, FPC], MT)
            eshift = cst.tile([P, 1], F32)
            nc.vector.memset(eshift[:], -12.0)
            nc.sync.dma_start(bv_sb[:], bv.ap()[None, :])

            # persistent phase A->B products
            qk_sb = pAB.tile([P, 2 * FT, S], MT)          # slots: Q ft 0..FT-1, K ft FT..2FT-1
            v_sb = pAB.tile([P, KC, HPC, 66], MT)          # col 64 = ones

            nc.vector.memset(v_sb[:, :, :, 64:65], 1.0)

            PSW = max(NQH, 512)  # psum tag width (fp32 elems per partition)

            # ---------------- Phase A: projections + rope ----------------
            # rotate_half is applied on-device: qpre (pre-RoPE Q/K) is copied
            # PSUM->SBUF fp16 on the (otherwise idle) scalar engine, then one
            # 128-contraction permutation matmul with prot produces the
            # rotated branch -- replacing the second full 1024-contraction
            # projection of the rotated-twin scheme. The perm matmul is
            # emitted one tile late so the PE never waits on the copy.
            with (
                tc.tile_pool(name="pA", bufs=1) as pA,
                tc.tile_pool(name="pAw", bufs=2) as pAw,
                tc.tile_pool(name="psA", bufs=2, space="PSUM") as ppsA,
            ):
                def emit_rot(st):
                    qpre, brcol, sin_ap, tca, dst = st
                    pb = ppsA.tile([P, NTA], F32, tag="pb", name="pb")
                    nc.tensor.matmul(pb[:], prot_sb[:], qpre[:],
                                     start=True, stop=True)
                    tcb = pAw.tile([P, NTA], F32, tag="tcb")
                    nc.vector.scalar_tensor_tensor(
                        tcb[:], pb[:], b4_sb[:, brcol:brcol + 1], sin_ap,
                        op0=mybir.AluOpType.add, op1=mybir.AluOpType.mult)
                    nc.vector.tensor_add(dst, tca[:], tcb[:])

                pend = None
                for half in range(2):
                    tsl = slice(half * NQH, (half + 1) * NQH)
                    h_sb = pA.tile([P, DC, NQH], MT, tag="hT", bufs=2)
                    nc.sync.dma_start(h_sb[:], hT_r[:, :, tsl])
                    cos_sb = pA.tile([P, NQH], F32, tag="cos", bufs=2)
                    nc.sync.dma_start(cos_sb[:], cosr.ap()[:, tsl])
                    sin_sb = pA.tile([P, NQH], F32, tag="sin", bufs=2)
                    nc.sync.dma_start(sin_sb[:], sinr.ap()[:, tsl])

                    # Q/K -> qk_sb
                    for qk in range(2):            # 0 = Q, 1 = K
                        for ft in range(FT):
                            fcol = qk * FPC + ft * P
                            wa = pAw.tile([P, DC, P], MT, tag="wA")
                            nc.sync.dma_start(wa[:], w4_r[:, :, fcol:fcol + P])
                            bcol = (qk * 2 * FPC + ft * P) // P
                            brcol = bcol + FPC // P
                            for tq in range(NQH // NTA):
                                qsl = slice(tq * NTA, (tq + 1) * NTA)
                                pa = ppsA.tile([P, NTA], F32, tag="pa", name="pa")
                                for dc in range(DC):
                                    nc.tensor.matmul(pa[:], wa[:, dc], h_sb[:, dc, qsl],
                                                     start=(dc == 0), stop=(dc == DC - 1))
                                qpre = pAw.tile([P, NTA], MT, tag="qpre")
                                nc.scalar.copy(qpre[:], pa[:])
                                tca = pAw.tile([P, NTA], F32, tag="tca")
                                nc.vector.scalar_tensor_tensor(
                                    tca[:], pa[:], b4_sb[:, bcol:bcol + 1], cos_sb[:, qsl],
                                    op0=mybir.AluOpType.add, op1=mybir.AluOpType.mult)
                                dst = qk_sb[:, qk * FT + ft, half * NQH + tq * NTA:
                                            half * NQH + (tq + 1) * NTA]
                                if pend is not None:
                                    emit_rot(pend)
                                pend = (qpre, brcol, sin_sb[:, qsl], tca, dst)

                    # V for this half: t-tiles within half
                    wvs = pA.tile([P, DC, FPC], MT, tag="wV")
                    nc.sync.dma_start(wvs[:], wv_r)
                    for tt in range(NQH // P):
                        gt = half * (NQH // P) + tt            # global t-tile = k-chunk
                        pv = ppsA.tile([P, FPC], F32, tag="pv", name="pv")
                        for dc in range(DC):
                            nc.tensor.matmul(pv[:], h_sb[:, dc, tt * P:(tt + 1) * P],
                                             wvs[:, dc], start=(dc == 0), stop=False)
                        nc.tensor.matmul(pv[:], ones1[:], bv_sb[:], start=False, stop=True)
                        nc.vector.tensor_copy(v_sb[:, gt, :, 0:64], pv[:])
                emit_rot(pend)

            with tc.tile_pool(name="pBC", bufs=1) as pBC:
                ctxT = pBC.tile([P, FT, S], MT)            # normalized ctx, f-major
                ow_sb = pBC.tile([P, FT, D], MT)
                nc.sync.dma_start(ow_sb[:], ow_r)
                # whole exp(bias) slab resident in SBUF: [k-in-chunk, kc, q]
                eb_slab = pBC.tile([P, KC, S], F16)
                ebr = expbT.ap().rearrange("(kc p) q -> p kc q", p=P)
                for kc4 in range(0, KC, 4):
                    nc.sync.dma_start(eb_slab[:, kc4:kc4 + 4, :],
                                      ebr[:, kc4:kc4 + 4, :])

                # ---------------- Phase B: attention ----------------
                # q chunks of NQ=512; kc processed in pairs packed into one
                # [P, 2*NQ] psS tile so exp covers 2 kc per instruction.
                # psS[hi=0] single-buffered (its exp runs FIRST each pair, so
                # the scores->exp->scores chain is one exp long); psS[hi=1]
                # double-buffered (its exp runs second, off the chain).
                # PSUM banks: s0 2 + s1 4 + ctx 2 = 8.
                NQB = 2 * NQ  # kc-pair-packed width
                # Schraudolph approximate exp: exp(x-12) ~=
                # bitcast_f32(int32(A*x + B)); ~2% rms relative error, used on
                # a subset of kc-pairs to offload the scalar engine.
                SCHRA_A = 12102203.161561485          # 2^23 / ln(2)
                SCHRA_B = float(127 * (1 << 23) - 366393) - 12.0 * SCHRA_A
                OFF_PAIRS = (1, 3, 5)
                with (
                    tc.tile_pool(name="pB", bufs=2) as pB,
                    tc.tile_pool(name="psB", bufs=1, space="PSUM") as ppsB,
                ):
                    for hp in range(HPC // 2):
                        for qq in range(S // NQ):
                            qoff = qq * NQ
                            qsl = slice(qoff, qoff + NQ)
                            cps = []
                            for i in range(2):
                                ct = ppsB.tile([P, NQ], F32, tag=f"ctx{i}",
                                               name=f"ctx{i}")
                                cps.append(ct[:65, :])
                            prev_us = None
                            prev_p = -1
                            for p2 in range(KC // 2):
                                psS = [
                                    ppsB.tile([P, NQB], F32, tag="s0", name="psS0",
                                              bufs=1),
                                    ppsB.tile([P, NQB], F32, tag="s1", name="psS1",
                                              bufs=2),
                                ]
                                # scores: h0/h1 adjacent for row-group overlap
                                prev_mm = None
                                for half in range(2):
                                    kc = 2 * p2 + half
                                    csl = slice(half * NQ, (half + 1) * NQ)
                                    for hi in range(2):
                                        h = 2 * hp + hi
                                        base = 64 * (h % 2)
                                        ft = h // 2
                                        ksl = qk_sb[base:base + 64, FT + ft,
                                                    kc * P:(kc + 1) * P]
                                        qap = qk_sb[base:base + 64, ft, qsl]
                                        mm = nc.tensor.matmul(psS[hi][:, csl], ksl,
                                                              qap, start=True,
                                                              stop=True)
                                        if prev_mm is not None:
                                            add_dep_helper(
                                                mm.ins, prev_mm.ins, sync=False,
                                                reason="scores row-group pairing")
                                        prev_mm = mm
                                us = []
                                for hi in range(2):  # hi=0 first: on the chain
                                    u2 = pB.tile([P, NQB], F16, tag=f"u2{hi}")
                                    if hi == 1 and p2 in OFF_PAIRS:
                                        # Schraudolph approx exp on DVE
                                        # (int32 bit-trick), bias-multiply on
                                        # the idle gpsimd engine -- keeps the
                                        # scalar engine free for hi=0's exp.
                                        ui = pB.tile([P, NQB], I32, tag="ui")
                                        nc.vector.tensor_scalar(
                                            ui[:], psS[hi][:], SCHRA_A, SCHRA_B,
                                            op0=mybir.AluOpType.mult,
                                            op1=mybir.AluOpType.add)
                                        nc.gpsimd.tensor_mul(
                                            u2.rearrange("p (a b) -> p a b", a=2),
                                            ui.bitcast(F32).rearrange(
                                                "p (a b) -> p a b", a=2),
                                            eb_slab[:, 2 * p2:2 * p2 + 2, qsl])
                                    else:
                                        u_sb = pB.tile([P, NQB], F16,
                                                       tag=f"u{hi}")
                                        nc.scalar.activation(u_sb[:], psS[hi][:],
                                                             AF.Exp,
                                                             bias=eshift[:])
                                        nc.vector.tensor_mul(
                                            u2.rearrange("p (a b) -> p a b", a=2),
                                            u_sb.rearrange("p (a b) -> p a b", a=2),
                                            eb_slab[:, 2 * p2:2 * p2 + 2, qsl])
                                    us.append(u2)
                                # software-pipeline: PV lags one kc-pair so PE
                                # never waits on this cycle's exp/mult
                                if prev_us is not None:
                                    for half in range(2):
                                        kc = 2 * prev_p + half
                                        csl = slice(half * NQ, (half + 1) * NQ)
                                        for hi in range(2):
                                            h = 2 * hp + hi
                                            nc.tensor.matmul(
                                                cps[hi][:],
                                                v_sb[:, kc, h, 0:65],
                                                prev_us[hi][:, csl],
                                                start=(kc == 0), stop=False)
                                prev_us, prev_p = us, p2
                            for half in range(2):
                                kc = 2 * prev_p + half
                                csl = slice(half * NQ, (half + 1) * NQ)
                                for hi in range(2):
                                    h = 2 * hp + hi
                                    nc.tensor.matmul(cps[hi][:],
                                                     v_sb[:, kc, h, 0:65],
                                                     prev_us[hi][:, csl],
                                                     start=False,
                                                     stop=(half == 1))
                            # finalize: evacuate ctx PSUM -> SBUF with two
                            # quick scalar copies (frees the ctx banks for the
                            # next chunk's PV immediately); the reciprocal /
                            # broadcast / normalize then run lazily from SBUF
                            # in the next chunk's shadow.
                            cus = []
                            for hi in range(2):
                                cu = pB.tile([65, NQ], F32, tag=f"cu{hi}")
                                if hi == 0:
                                    nc.scalar.copy(cu[:], cps[hi][:])
                                else:
                                    nc.vector.tensor_copy(cu[:], cps[hi][:])
                                cus.append(cu)
                            rscrs, rsqs, rrecs, rscr2s, rbs = [], [], [], [], []
                            for hi in range(2):
                                rscr = dpool.tile([NQ], F32)
                                nc.gpsimd.dma_start(rscr[None, :],
                                                    cus[hi][64:65, :])
                                rscrs.append(rscr)
                            for hi in range(2):
                                rsq = pB.tile([32, NQ // 32], F32, tag=f"rsq{hi}")
                                nc.gpsimd.dma_start(
                                    rsq[:], rscrs[hi].rearrange("(a b) -> a b", a=32))
                                rsqs.append(rsq)
                            for hi in range(2):
                                rrec = pB.tile([32, NQ // 32], F32, tag=f"rrec{hi}")
                                nc.vector.reciprocal(rrec[:], rsqs[hi][:])
                                rrecs.append(rrec)
                            for hi in range(2):
                                rscr2 = dpool.tile([NQ], F32)
                                nc.gpsimd.dma_start(
                                    rscr2.rearrange("(a b) -> a b", a=32), rrecs[hi][:])
                                rscr2s.append(rscr2)
                            for hi in range(2):
                                rb = pB.tile([64, NQ], F32, tag=f"rb{hi}")
                                nc.gpsimd.dma_start(rb[:],
                                                    rscr2s[hi].partition_broadcast(64))
                                rbs.append(rb)
                            for hi in range(2):
                                h = 2 * hp + hi
                                base = 64 * (h % 2)
                                ft = h // 2
                                nc.vector.tensor_mul(
                                    ctxT[base:base + 64, ft, qsl],
                                    cus[hi][0:64, :], rbs[hi][:])

                # ---------------- Phase C: output projection ----------------
                with (
                    tc.tile_pool(name="pC", bufs=4) as pC,
                    tc.tile_pool(name="psC", bufs=4, space="PSUM") as ppsC,
                ):
                    g = 0
                    for tq in range(TT4):
                        for ot in range(D // P):
                            tsl = slice(tq * NT, (tq + 1) * NT)
                            po = ppsC.tile([P, NT], F32, tag="po", name="po")
                            for fc in range(FT):
                                nc.tensor.matmul(po[:],
                                                 ow_sb[:, fc, ot * P:(ot + 1) * P],
                                                 ctxT[:, fc, tsl],
                                                 start=(fc == 0), stop=(fc == FT - 1))
                            o_sb = pC.tile([P, NT], F32, tag="oT")
                            if g % 2 == 0:
                                nc.scalar.copy(o_sb[:], po[:])
                            else:
                                nc.vector.tensor_copy(o_sb[:], po[:])
                            nc.sync.dma_start(outT.ap()[ot * P:(ot + 1) * P, tsl],
                                              o_sb[:])
                            g += 1

    nc.compile()
    return nc


def make_core_inputs(hidden_states, attention_bias, rope_cos, rope_sin, head_mask,
                     qkv_w, qkv_b, o_w, S=2048, D=1024, fast=True):
    """Host-side sharding + layout preparation. Returns list of 8 input dicts."""
    f32 = np.float32
    mt = np.float16 if fast else np.float32
    f16 = np.float16
    hidden_states = np.asarray(hidden_states, f32)
    attention_bias = np.asarray(attention_bias, f32)
    rope_cos = np.asarray(rope_cos, f32)
    rope_sin = np.asarray(rope_sin, f32)
    head_mask = np.asarray(head_mask, f32).reshape(-1)
    qkv_w = np.asarray(qkv_w, f32)
    qkv_b = np.asarray(qkv_b, f32)
    o_w = np.asarray(o_w, f32)

    B = hidden_states.shape[0]
    FPC = HPC * 64
    F = H * 64  # qkv feature dim (row-section size of qkv_w)

    def rot_rows(w):
        # rows indexed by f = hl*64 + d; rot(q)[d] = -q[d+32] (d<32) else q[d-32]
        w = w.reshape(HPC, 64, -1) if w.ndim == 2 else w.reshape(HPC, 64)
        lo, hi = w[:, 0:32], w[:, 32:64]
        out = np.concatenate([-hi, lo], axis=1)
        return out.reshape(HPC * 64, -1) if out.ndim == 3 else out.reshape(HPC * 64)

    cos_t = rope_cos[0, :, 0, :].T.astype(f32)     # [64, S]
    sin_t = rope_sin[0, :, 0, :].T.astype(f32)
    cosr = np.concatenate([cos_t, cos_t], axis=0)  # [128, S]
    sinr = np.concatenate([sin_t, sin_t], axis=0)

    # on-device rotate_half permutation: out[fo] = sum_fi prot[fi, fo] * in[fi]
    protm = np.zeros((128, 128), np.float32)
    for blk in (0, 64):
        for dd in range(32):
            protm[blk + 32 + dd, blk + dd] = -1.0
            protm[blk + dd, blk + 32 + dd] = 1.0

    in_maps = []
    for c in range(8):
        b, g = divmod(c, G)
        fs = slice(g * FPC, (g + 1) * FPC)
        wq = qkv_w[F * 0:F * 1][fs]
        wk = qkv_w[F * 1:F * 2][fs]
        wv = qkv_w[F * 2:F * 3][fs].copy()
        bq = qkv_b[F * 0:F * 1][fs]
        bk = qkv_b[F * 1:F * 2][fs]
        bvv = qkv_b[F * 2:F * 3][fs].copy()
        mask = head_mask[g * HPC:(g + 1) * HPC]
        wv *= np.repeat(mask, 64)[:, None]
        bvv *= np.repeat(mask, 64)
        bqr = rot_rows(bq)
        bkr = rot_rows(bk)
        w4 = np.concatenate([wq.T, wk.T], axis=1)  # [D, 2*FPC]
        b4 = np.concatenate([bq, bqr, bk, bkr])
        bT = np.ascontiguousarray(attention_bias[b, 0].T)
        m = {
            "hT": np.ascontiguousarray(hidden_states[b].T).astype(mt),
            "w4": np.ascontiguousarray(w4).astype(mt),
            "b4": np.ascontiguousarray(b4),
            "wvT": np.ascontiguousarray(wv.T).astype(mt),
            "bv": np.ascontiguousarray(bvv).astype(mt),
            "cosr": np.ascontiguousarray(cosr),
            "sinr": np.ascontiguousarray(sinr),
            "prot": protm.astype(mt),
            "owT": np.ascontiguousarray(o_w[:, g * FPC:(g + 1) * FPC].T).astype(mt),
        }
        if fast:
            m["expbT"] = np.exp(bT).astype(f16)
        else:
            m["biasT"] = bT
        in_maps.append(m)
    return in_maps


def kernel(hidden_states, attention_bias, rope_cos, rope_sin, head_mask,
           qkv_w, qkv_b, o_w, o_b, **_unused):
    from concourse.bass_utils import run_bass_kernel_spmd

    B, S, D = hidden_states.shape
    fast = _CACHE.get("fast", True)
    if "nc" not in _CACHE:
        _CACHE["nc"] = build_nc(S=S, D=D, fast=fast)
    nc = _CACHE["nc"]

    in_maps = make_core_inputs(hidden_states, attention_bias, rope_cos, rope_sin,
                               head_mask, qkv_w, qkv_b, o_w, S=S, D=D, fast=fast)
    res = run_bass_kernel_spmd(nc, in_maps, list(range(8)))
    _CACHE["last_results"] = res

    o_b = np.asarray(o_b, np.float32)
    out = np.empty((B, S, D), np.float32)
    for b in range(B):
        acc = res.results[2 * b]["outT"].T + res.results[2 * b + 1]["outT"].T
        out[b] = acc + o_b[None, :]
    return out



# revision 23
# speedup vs baseline: 1.0128x; 1.0128x over previous
"""Trainium2 Bass kernel for nn_Attention_8143257993917.

Multi-head attention (packed QKV + RoPE + additive bias + softmax + head_mask
+ o_proj), B=4, S=2048, D=1024, H=16 heads, fp32 I/O.

Sharding: 8 cores = 4 batches x 2 head-groups (tensor-parallel over heads).
Core c handles batch b = c // 2 and heads g*8..g*8+8 with g = c % 2.
Each core computes a partial output (its heads' contribution through o_proj);
the host sums the two partials per batch and adds o_b.

Device-side design (per core, fast mode):
- Everything runs in "transposed" feature-major layouts so the big score /
  probability matrices never need an on-chip transpose:
    Q_T, K_T: [f, t] (f = head*64+d on partitions): out[f,t] = wT[d,f].T @ hT.
    RoPE: q' = (q + bq) * cos + (rot(q) + rot(bq)) * sin, where the rotated
      branch comes from a SECOND projection with host-prerotated weights
      (rotate_half is a row permutation+sign of W, so it folds into weights).
    V: [t, f] natural layout, so V chunks [k=128, d=64] are directly the
      stationary operand of the PV matmul. A ones-column appended to V makes
      the PV matmul also produce the softmax denominators (row 64 of ctx).
    scores S_T[k, q] = K_T_chunk.T @ Q_T (contraction d=64), fp16 operands,
      fp32 PSUM accumulate.
    bias: exp(S+b) = exp(S)*exp(b); exp(bias) is precomputed on the host in
      fp16 and applied as one elementwise multiply on VectorE (removes 512
      identity-matmul bias adds from the PE).
    exp on ScalarE (PSUM -> SBUF) with a constant -12 shift (softmax is
      shift-invariant; keeps exp outputs inside fp16 range).
    PV is software-pipelined one k-chunk behind scores/exp/mult so the PE
      never waits on the current chunk's ScalarE/VectorE results.
    softmax denominators: exact VectorE reciprocal on a [32, NQH/32] reshape
      (via a small DRAM round-trip on the gpsimd DMA queues, which also
      broadcasts 1/r across 64 partitions); one TT multiply normalizes ctx
      and moves it PSUM -> SBUF.
    head_mask is folded into the V projection weights/bias on the host.
    o_proj: out_T[o, t] = sum_f o_wT[f, o] * ctx_T[f, t], fp16 operands.
  Matmul dtype is fp16 rather than bf16: same PE throughput, ~8x lower
  quantization error (all value ranges verified to fit fp16 comfortably).
  fp32 matmuls on TRN2 lower to LOW_HIGH double-pass + 2 cycles/column
  streaming (~5x slower than fp16), hence the fp16 datapath with fp32
  accumulation; measured end-to-end relative error vs the fp32 reference
  is ~1.3e-3.
"""

import sys

sys.path.insert(0, "/opt/trn_rl_repo")

import numpy as np

_CACHE = {}

H = 16
HPC = 8  # heads per core
G = 2  # head groups


def build_nc(S=2048, D=1024, fast=True):
    """Build + compile the per-core Bass program (same program on all cores)."""
    import concourse.bass as bass
    from concourse import bacc
    import concourse.mybir as mybir
    import concourse.tile as tile
    from concourse.masks import make_identity
    from concourse.tile_rust import add_dep_helper

    F32 = mybir.dt.float32
    BF16 = mybir.dt.bfloat16
    F16 = mybir.dt.float16
    I32 = mybir.dt.int32
    MT = F16 if fast else F32      # matmul operand dtype
    AF = mybir.ActivationFunctionType

    P = 128
    DC = D // P          # d chunks (contraction for projections)
    KC = S // P          # k chunks (scores contraction)
    NQH = S // 2         # q-half size
    NQ = min(512, NQH)   # matmul free-dim chunk
    NQC = NQH // NQ      # chunks per q-half
    FPC = HPC * 64       # features per core (= 512)
    FT = FPC // P        # f-tiles per tensor (= 4)
    NT = min(512, S)     # phase C t-chunk
    TT4 = S // NT
    NTA = min(512, NQH)  # phase A t-chunk

    nc = bacc.Bacc("TRN2", target_bir_lowering=False, debug=False, num_devices=8)

    hT = nc.dram_tensor("hT", [D, S], MT, kind="ExternalInput")
    w4 = nc.dram_tensor("w4", [D, 2 * FPC], MT, kind="ExternalInput")
    b4 = nc.dram_tensor("b4", [4 * FPC], F32, kind="ExternalInput")
    prot = nc.dram_tensor("prot", [P, P], MT, kind="ExternalInput")
    wvT = nc.dram_tensor("wvT", [D, FPC], MT, kind="ExternalInput")
    bv = nc.dram_tensor("bv", [FPC], MT, kind="ExternalInput")
    cosr = nc.dram_tensor("cosr", [P, S], F32, kind="ExternalInput")
    sinr = nc.dram_tensor("sinr", [P, S], F32, kind="ExternalInput")
    if fast:
        expbT = nc.dram_tensor("expbT", [S, S], F16, kind="ExternalInput")
    else:
        biasT = nc.dram_tensor("biasT", [S, S], F32, kind="ExternalInput")
    owT = nc.dram_tensor("owT", [FPC, D], MT, kind="ExternalInput")
    outT = nc.dram_tensor("outT", [D, S], F32, kind="ExternalOutput")

    hT_r = hT.ap().rearrange("(o p) t -> p o t", p=P)
    w4_r = w4.ap().rearrange("(o p) f -> p o f", p=P)
    wv_r = wvT.ap().rearrange("(o p) f -> p o f", p=P)
    ow_r = owT.ap().rearrange("(o p) f -> p o f", p=P)
    b4_r = b4.ap().rearrange("(o p) -> p o", p=P)

    with tile.TileContext(nc) as tc:
        with (
            tc.tile_pool(name="cst", bufs=1) as cst,
            tc.tile_pool(name="pAB", bufs=1) as pAB,
            tc.tile_pool(name="dram", bufs=4, space="DRAM") as dpool,
        ):
            ident = cst.tile([P, P], F32)
            make_identity(nc, ident)
            prot_sb = cst.tile([P, P], MT)
            nc.sync.dma_start(prot_sb[:], prot.ap())
            ones1 = cst.tile([1, P], MT)
            nc.vector.memset(ones1[:], 1.0)
            b4_sb = cst.tile([P, 4 * FPC // P], F32)
            nc.sync.dma_start(b4_sb[:], b4_r)
            bv_sb = cst.tile([1, FPC], MT)
            eshift = cst.tile([P, 1], F32)
            nc.vector.memset(eshift[:], -12.0)
            nc.sync.dma_start(bv_sb[:], bv.ap()[None, :])

            # persistent phase A->B products
            qk_sb = pAB.tile([P, 2 * FT, S], MT)          # slots: Q ft 0..FT-1, K ft FT..2FT-1
            v_sb = pAB.tile([P, KC, HPC, 66], MT)          # col 64 = ones

            nc.vector.memset(v_sb[:, :, :, 64:65], 1.0)

            PSW = max(NQH, 512)  # psum tag width (fp32 elems per partition)

            # ---------------- Phase A: projections + rope ----------------
            # rotate_half is applied on-device: qpre (pre-RoPE Q/K) is copied
            # PSUM->SBUF fp16 on the (otherwise idle) scalar engine, then one
            # 128-contraction permutation matmul with prot produces the
            # rotated branch -- replacing the second full 1024-contraction
            # projection of the rotated-twin scheme. The perm matmul is
            # emitted one tile late so the PE never waits on the copy.
            with (
                tc.tile_pool(name="pA", bufs=1) as pA,
                tc.tile_pool(name="pAw", bufs=2) as pAw,
                tc.tile_pool(name="psA", bufs=2, space="PSUM") as ppsA,
            ):
                def emit_rot(st):
                    qpre, brcol, sin_ap, tca, dst = st
                    pb = ppsA.tile([P, NTA], F32, tag="pb", name="pb")
                    nc.tensor.matmul(pb[:], prot_sb[:], qpre[:],
                                     start=True, stop=True)
                    tcb = pAw.tile([P, NTA], F32, tag="tcb")
                    nc.vector.scalar_tensor_tensor(
                        tcb[:], pb[:], b4_sb[:, brcol:brcol + 1], sin_ap,
                        op0=mybir.AluOpType.add, op1=mybir.AluOpType.mult)
                    nc.vector.tensor_add(dst, tca[:], tcb[:])

                pend = None
                for half in range(2):
                    tsl = slice(half * NQH, (half + 1) * NQH)
                    h_sb = pA.tile([P, DC, NQH], MT, tag="hT", bufs=2)
                    nc.sync.dma_start(h_sb[:], hT_r[:, :, tsl])
                    cos_sb = pA.tile([P, NQH], F32, tag="cos", bufs=2)
                    nc.sync.dma_start(cos_sb[:], cosr.ap()[:, tsl])
                    sin_sb = pA.tile([P, NQH], F32, tag="sin", bufs=2)
                    nc.sync.dma_start(sin_sb[:], sinr.ap()[:, tsl])

                    # Q/K -> qk_sb
                    for qk in range(2):            # 0 = Q, 1 = K
                        for ft in range(FT):
                            fcol = qk * FPC + ft * P
                            wa = pAw.tile([P, DC, P], MT, tag="wA")
                            nc.sync.dma_start(wa[:], w4_r[:, :, fcol:fcol + P])
                            bcol = (qk * 2 * FPC + ft * P) // P
                            brcol = bcol + FPC // P
                            for tq in range(NQH // NTA):
                                qsl = slice(tq * NTA, (tq + 1) * NTA)
                                pa = ppsA.tile([P, NTA], F32, tag="pa", name="pa")
                                for dc in range(DC):
                                    nc.tensor.matmul(pa[:], wa[:, dc], h_sb[:, dc, qsl],
                                                     start=(dc == 0), stop=(dc == DC - 1))
                                qpre = pAw.tile([P, NTA], MT, tag="qpre")
                                nc.scalar.copy(qpre[:], pa[:])
                                tca = pAw.tile([P, NTA], F32, tag="tca")
                                nc.vector.scalar_tensor_tensor(
                                    tca[:], pa[:], b4_sb[:, bcol:bcol + 1], cos_sb[:, qsl],
                                    op0=mybir.AluOpType.add, op1=mybir.AluOpType.mult)
                                dst = qk_sb[:, qk * FT + ft, half * NQH + tq * NTA:
                                            half * NQH + (tq + 1) * NTA]
                                if pend is not None:
                                    emit_rot(pend)
                                pend = (qpre, brcol, sin_sb[:, qsl], tca, dst)

                    # V for this half: t-tiles within half
                    wvs = pA.tile([P, DC, FPC], MT, tag="wV")
                    nc.sync.dma_start(wvs[:], wv_r)
                    for tt in range(NQH // P):
                        gt = half * (NQH // P) + tt            # global t-tile = k-chunk
                        pv = ppsA.tile([P, FPC], F32, tag="pv", name="pv")
                        for dc in range(DC):
                            nc.tensor.matmul(pv[:], h_sb[:, dc, tt * P:(tt + 1) * P],
                                             wvs[:, dc], start=(dc == 0), stop=False)
                        nc.tensor.matmul(pv[:], ones1[:], bv_sb[:], start=False, stop=True)
                        nc.vector.tensor_copy(v_sb[:, gt, :, 0:64], pv[:])
                emit_rot(pend)

            with tc.tile_pool(name="pBC", bufs=1) as pBC:
                ctxT = pBC.tile([P, FT, S], MT)            # normalized ctx, f-major
                ow_sb = pBC.tile([P, FT, D], MT)
                nc.sync.dma_start(ow_sb[:], ow_r)
                # whole exp(bias) slab resident in SBUF: [k-in-chunk, kc, q]
                eb_slab = pBC.tile([P, KC, S], F16)
                ebr = expbT.ap().rearrange("(kc p) q -> p kc q", p=P)
                for kc4 in range(0, KC, 4):
                    nc.sync.dma_start(eb_slab[:, kc4:kc4 + 4, :],
                                      ebr[:, kc4:kc4 + 4, :])

                # ---------------- Phase B: attention ----------------
                # q chunks of NQ=512; kc processed in pairs packed into one
                # [P, 2*NQ] psS tile so exp covers 2 kc per instruction.
                # psS[hi=0] single-buffered (its exp runs FIRST each pair, so
                # the scores->exp->scores chain is one exp long); psS[hi=1]
                # double-buffered (its exp runs second, off the chain).
                # PSUM banks: s0 2 + s1 4 + ctx 2 = 8.
                NQB = 2 * NQ  # kc-pair-packed width
                # Schraudolph approximate exp: exp(x-12) ~=
                # bitcast_f32(int32(A*x + B)); ~2% rms relative error, used on
                # a subset of kc-pairs to offload the scalar engine.
                SCHRA_A = 12102203.161561485          # 2^23 / ln(2)
                SCHRA_B = float(127 * (1 << 23) - 366393) - 12.0 * SCHRA_A
                OFF_PAIRS = (1, 3, 5)
                with (
                    tc.tile_pool(name="pB", bufs=2) as pB,
                    tc.tile_pool(name="psB", bufs=1, space="PSUM") as ppsB,
                ):
                    for hp in range(HPC // 2):
                        for qq in range(S // NQ):
                            qoff = qq * NQ
                            qsl = slice(qoff, qoff + NQ)
                            cps = []
                            for i in range(2):
                                ct = ppsB.tile([P, NQ], F32, tag=f"ctx{i}",
                                               name=f"ctx{i}")
                                cps.append(ct[:65, :])
                            def emit_pv(pp, uu, last=False):
                                for half in range(2):
                                    kc = 2 * pp + half
                                    csl = slice(half * NQ, (half + 1) * NQ)
                                    for hi in range(2):
                                        h = 2 * hp + hi
                                        nc.tensor.matmul(
                                            cps[hi][:],
                                            v_sb[:, kc, h, 0:65],
                                            uu[hi][:, csl],
                                            start=(kc == 0),
                                            stop=(last and half == 1))

                            pending = []
                            for p2 in range(KC // 2):
                                psS = [
                                    ppsB.tile([P, NQB], F32, tag="s0", name="psS0",
                                              bufs=1),
                                    ppsB.tile([P, NQB], F32, tag="s1", name="psS1",
                                              bufs=2),
                                ]
                                # scores: h0/h1 adjacent for row-group overlap
                                prev_mm = None
                                for half in range(2):
                                    kc = 2 * p2 + half
                                    csl = slice(half * NQ, (half + 1) * NQ)
                                    for hi in range(2):
                                        h = 2 * hp + hi
                                        base = 64 * (h % 2)
                                        ft = h // 2
                                        ksl = qk_sb[base:base + 64, FT + ft,
                                                    kc * P:(kc + 1) * P]
                                        qap = qk_sb[base:base + 64, ft, qsl]
                                        mm = nc.tensor.matmul(psS[hi][:, csl], ksl,
                                                              qap, start=True,
                                                              stop=True)
                                        if prev_mm is not None:
                                            add_dep_helper(
                                                mm.ins, prev_mm.ins, sync=False,
                                                reason="scores row-group pairing")
                                        prev_mm = mm
                                off = p2 in OFF_PAIRS
                                u2s = [pB.tile([P, NQB], F16, tag=f"u2{hi}",
                                               bufs=3, name=f"u2{hi}")
                                       for hi in range(2)]
                                if off:
                                    # Schraudolph approx exp on DVE (int32
                                    # bit-trick), bias-multiply on the idle
                                    # gpsimd engine -- relieves the scalar
                                    # engine. Emitted before exp(hi0) so the
                                    # long DVE->gpsimd chain starts early.
                                    ui = pB.tile([P, NQB], I32, tag="ui")
                                    nc.vector.tensor_scalar(
                                        ui[:], psS[1][:], SCHRA_A, SCHRA_B,
                                        op0=mybir.AluOpType.mult,
                                        op1=mybir.AluOpType.add)
                                    nc.gpsimd.tensor_mul(
                                        u2s[1].rearrange("p (a b) -> p a b", a=2),
                                        ui.bitcast(F32).rearrange(
                                            "p (a b) -> p a b", a=2),
                                        eb_slab[:, 2 * p2:2 * p2 + 2, qsl])
                                for hi in range(2):  # hi=0 first: on the chain
                                    if hi == 1 and off:
                                        continue
                                    u_sb = pB.tile([P, NQB], F16, tag=f"u{hi}")
                                    nc.scalar.activation(u_sb[:], psS[hi][:],
                                                         AF.Exp, bias=eshift[:])
                                    nc.vector.tensor_mul(
                                        u2s[hi].rearrange("p (a b) -> p a b", a=2),
                                        u_sb.rearrange("p (a b) -> p a b", a=2),
                                        eb_slab[:, 2 * p2:2 * p2 + 2, qsl])
                                # software-pipeline: PV lags two kc-pairs so
                                # the PE never waits on the exp/mult chain
                                # (the approx path has ~4us of latency).
                                pending.append((p2, u2s))
                                if len(pending) > 2:
                                    pp, uu = pending.pop(0)
                                    emit_pv(pp, uu)
                            while pending:
                                pp, uu = pending.pop(0)
                                emit_pv(pp, uu, last=not pending)
                            # finalize: evacuate ctx PSUM -> SBUF with two
                            # quick scalar copies (frees the ctx banks for the
                            # next chunk's PV immediately); the reciprocal /
                            # broadcast / normalize then run lazily from SBUF
                            # in the next chunk's shadow.
                            cus = []
                            for hi in range(2):
                                cu = pB.tile([65, NQ], F32, tag=f"cu{hi}")
                                if hi == 0:
                                    nc.scalar.copy(cu[:], cps[hi][:])
                                else:
                                    nc.vector.tensor_copy(cu[:], cps[hi][:])
                                cus.append(cu)
                            rscrs, rsqs, rrecs, rscr2s, rbs = [], [], [], [], []
                            for hi in range(2):
                                rscr = dpool.tile([NQ], F32)
                                nc.gpsimd.dma_start(rscr[None, :],
                                                    cus[hi][64:65, :])
                                rscrs.append(rscr)
                            for hi in range(2):
                                rsq = pB.tile([32, NQ // 32], F32, tag=f"rsq{hi}")
                                nc.gpsimd.dma_start(
                                    rsq[:], rscrs[hi].rearrange("(a b) -> a b", a=32))
                                rsqs.append(rsq)
                            for hi in range(2):
                                rrec = pB.tile([32, NQ // 32], F32, tag=f"rrec{hi}")
                                nc.vector.reciprocal(rrec[:], rsqs[hi][:])
                                rrecs.append(rrec)
                            for hi in range(2):
                                rscr2 = dpool.tile([NQ], F32)
                                nc.gpsimd.dma_start(
                                    rscr2.rearrange("(a b) -> a b", a=32), rrecs[hi][:])
                                rscr2s.append(rscr2)
                            for hi in range(2):
                                rb = pB.tile([64, NQ], F32, tag=f"rb{hi}")
                                nc.gpsimd.dma_start(rb[:],
                                                    rscr2s[hi].partition_broadcast(64))
                                rbs.append(rb)
                            for hi in range(2):
                                h = 2 * hp + hi
                                base = 64 * (h % 2)
                                ft = h // 2
                                nc.vector.tensor_mul(
                                    ctxT[base:base + 64, ft, qsl],
                                    cus[hi][0:64, :], rbs[hi][:])

                # ---------------- Phase C: output projection ----------------
                with (
                    tc.tile_pool(name="pC", bufs=4) as pC,
                    tc.tile_pool(name="psC", bufs=4, space="PSUM") as ppsC,
                ):
                    g = 0
                    for tq in range(TT4):
                        for ot in range(D // P):
                            tsl = slice(tq * NT, (tq + 1) * NT)
                            po = ppsC.tile([P, NT], F32, tag="po", name="po")
                            for fc in range(FT):
                                nc.tensor.matmul(po[:],
                                                 ow_sb[:, fc, ot * P:(ot + 1) * P],
                                                 ctxT[:, fc, tsl],
                                                 start=(fc == 0), stop=(fc == FT - 1))
                            o_sb = pC.tile([P, NT], F32, tag="oT")
                            if g % 2 == 0:
                                nc.scalar.copy(o_sb[:], po[:])
                            else:
                                nc.vector.tensor_copy(o_sb[:], po[:])
                            nc.sync.dma_start(outT.ap()[ot * P:(ot + 1) * P, tsl],
                                              o_sb[:])
                            g += 1

    nc.compile()
    return nc


def make_core_inputs(hidden_states, attention_bias, rope_cos, rope_sin, head_mask,
                     qkv_w, qkv_b, o_w, S=2048, D=1024, fast=True):
    """Host-side sharding + layout preparation. Returns list of 8 input dicts."""
    f32 = np.float32
    mt = np.float16 if fast else np.float32
    f16 = np.float16
    hidden_states = np.asarray(hidden_states, f32)
    attention_bias = np.asarray(attention_bias, f32)
    rope_cos = np.asarray(rope_cos, f32)
    rope_sin = np.asarray(rope_sin, f32)
    head_mask = np.asarray(head_mask, f32).reshape(-1)
    qkv_w = np.asarray(qkv_w, f32)
    qkv_b = np.asarray(qkv_b, f32)
    o_w = np.asarray(o_w, f32)

    B = hidden_states.shape[0]
    FPC = HPC * 64
    F = H * 64  # qkv feature dim (row-section size of qkv_w)

    def rot_rows(w):
        # rows indexed by f = hl*64 + d; rot(q)[d] = -q[d+32] (d<32) else q[d-32]
        w = w.reshape(HPC, 64, -1) if w.ndim == 2 else w.reshape(HPC, 64)
        lo, hi = w[:, 0:32], w[:, 32:64]
        out = np.concatenate([-hi, lo], axis=1)
        return out.reshape(HPC * 64, -1) if out.ndim == 3 else out.reshape(HPC * 64)

    cos_t = rope_cos[0, :, 0, :].T.astype(f32)     # [64, S]
    sin_t = rope_sin[0, :, 0, :].T.astype(f32)
    cosr = np.concatenate([cos_t, cos_t], axis=0)  # [128, S]
    sinr = np.concatenate([sin_t, sin_t], axis=0)

    # on-device rotate_half permutation: out[fo] = sum_fi prot[fi, fo] * in[fi]
    protm = np.zeros((128, 128), np.float32)
    for blk in (0, 64):
        for dd in range(32):
            protm[blk + 32 + dd, blk + dd] = -1.0
            protm[blk + dd, blk + 32 + dd] = 1.0

    in_maps = []
    for c in range(8):
        b, g = divmod(c, G)
        fs = slice(g * FPC, (g + 1) * FPC)
        wq = qkv_w[F * 0:F * 1][fs]
        wk = qkv_w[F * 1:F * 2][fs]
        wv = qkv_w[F * 2:F * 3][fs].copy()
        bq = qkv_b[F * 0:F * 1][fs]
        bk = qkv_b[F * 1:F * 2][fs]
        bvv = qkv_b[F * 2:F * 3][fs].copy()
        mask = head_mask[g * HPC:(g + 1) * HPC]
        wv *= np.repeat(mask, 64)[:, None]
        bvv *= np.repeat(mask, 64)
        bqr = rot_rows(bq)
        bkr = rot_rows(bk)
        w4 = np.concatenate([wq.T, wk.T], axis=1)  # [D, 2*FPC]
        b4 = np.concatenate([bq, bqr, bk, bkr])
        bT = np.ascontiguousarray(attention_bias[b, 0].T)
        m = {
            "hT": np.ascontiguousarray(hidden_states[b].T).astype(mt),
            "w4": np.ascontiguousarray(w4).astype(mt),
            "b4": np.ascontiguousarray(b4),
            "wvT": np.ascontiguousarray(wv.T).astype(mt),
            "bv": np.ascontiguousarray(bvv).astype(mt),
            "cosr": np.ascontiguousarray(cosr),
            "sinr": np.ascontiguousarray(sinr),
            "prot": protm.astype(mt),
            "owT": np.ascontiguousarray(o_w[:, g * FPC:(g + 1) * FPC].T).astype(mt),
        }
        if fast:
            m["expbT"] = np.exp(bT).astype(f16)
        else:
            m["biasT"] = bT
        in_maps.append(m)
    return in_maps


def kernel(hidden_states, attention_bias, rope_cos, rope_sin, head_mask,
           qkv_w, qkv_b, o_w, o_b, **_unused):
    from concourse.bass_utils import run_bass_kernel_spmd

    B, S, D = hidden_states.shape
    fast = _CACHE.get("fast", True)
    if "nc" not in _CACHE:
        _CACHE["nc"] = build_nc(S=S, D=D, fast=fast)
    nc = _CACHE["nc"]

    in_maps = make_core_inputs(hidden_states, attention_bias, rope_cos, rope_sin,
                               head_mask, qkv_w, qkv_b, o_w, S=S, D=D, fast=fast)
    res = run_bass_kernel_spmd(nc, in_maps, list(range(8)))
    _CACHE["last_results"] = res

    o_b = np.asarray(o_b, np.float32)
    out = np.empty((B, S, D), np.float32)
    for b in range(B):
        acc = res.results[2 * b]["outT"].T + res.results[2 * b + 1]["outT"].T
        out[b] = acc + o_b[None, :]
    return out



# revision 24
# speedup vs baseline: 1.2036x; 1.1884x over previous
"""Trainium2 Bass kernel for nn_Attention_8143257993917.

Multi-head attention (packed QKV + RoPE + additive bias + softmax + head_mask
+ o_proj), B=4, S=2048, D=1024, H=16 heads, fp32 I/O.

Sharding: 8 cores = 4 batches x 2 head-groups (tensor-parallel over heads).
Core c handles batch b = c // 2 and heads g*8..g*8+8 with g = c % 2.
Each core computes a partial output (its heads' contribution through o_proj);
the host sums the two partials per batch and adds o_b.

Device-side design (per core, fast mode):
- Everything runs in "transposed" feature-major layouts so the big score /
  probability matrices never need an on-chip transpose:
    Q_T, K_T: [f, t] (f = head*64+d on partitions): out[f,t] = wT[d,f].T @ hT.
    RoPE: q' = (q + bq) * cos + (rot(q) + rot(bq)) * sin, where the rotated
      branch comes from a SECOND projection with host-prerotated weights
      (rotate_half is a row permutation+sign of W, so it folds into weights).
    V: [t, f] natural layout, so V chunks [k=128, d=64] are directly the
      stationary operand of the PV matmul. A ones-column appended to V makes
      the PV matmul also produce the softmax denominators (row 64 of ctx).
    scores S_T[k, q] = K_T_chunk.T @ Q_T (contraction d=64), fp16 operands,
      fp32 PSUM accumulate.
    bias: exp(S+b) = exp(S)*exp(b); exp(bias) is precomputed on the host in
      fp16 and applied as one elementwise multiply on VectorE (removes 512
      identity-matmul bias adds from the PE).
    exp on ScalarE (PSUM -> SBUF) with a constant -12 shift (softmax is
      shift-invariant; keeps exp outputs inside fp16 range).
    PV is software-pipelined one k-chunk behind scores/exp/mult so the PE
      never waits on the current chunk's ScalarE/VectorE results.
    softmax denominators: exact VectorE reciprocal on a [32, NQH/32] reshape
      (via a small DRAM round-trip on the gpsimd DMA queues, which also
      broadcasts 1/r across 64 partitions); one TT multiply normalizes ctx
      and moves it PSUM -> SBUF.
    head_mask is folded into the V projection weights/bias on the host.
    o_proj: out_T[o, t] = sum_f o_wT[f, o] * ctx_T[f, t], fp16 operands.
  Matmul dtype is fp16 rather than bf16: same PE throughput, ~8x lower
  quantization error (all value ranges verified to fit fp16 comfortably).
  fp32 matmuls on TRN2 lower to LOW_HIGH double-pass + 2 cycles/column
  streaming (~5x slower than fp16), hence the fp16 datapath with fp32
  accumulation; measured end-to-end relative error vs the fp32 reference
  is ~1.3e-3.
"""

import sys

sys.path.insert(0, "/opt/trn_rl_repo")

import numpy as np

_CACHE = {}

H = 16
HPC = 8  # heads per core
G = 2  # head groups


def build_nc(S=2048, D=1024, fast=True):
    """Build + compile the per-core Bass program (same program on all cores)."""
    import concourse.bass as bass
    from concourse import bacc
    import concourse.mybir as mybir
    import concourse.tile as tile
    from concourse.masks import make_identity
    from concourse.tile_rust import add_dep_helper

    F32 = mybir.dt.float32
    BF16 = mybir.dt.bfloat16
    F16 = mybir.dt.float16
    I32 = mybir.dt.int32
    MT = F16 if fast else F32      # matmul operand dtype
    AF = mybir.ActivationFunctionType

    P = 128
    DC = D // P          # d chunks (contraction for projections)
    KC = S // P          # k chunks (scores contraction)
    NQH = S // 2         # q-half size
    NQ = min(512, NQH)   # matmul free-dim chunk
    NQC = NQH // NQ      # chunks per q-half
    FPC = HPC * 64       # features per core (= 512)
    FT = FPC // P        # f-tiles per tensor (= 4)
    NT = min(512, S)     # phase C t-chunk
    TT4 = S // NT
    NTA = min(512, NQH)  # phase A t-chunk

    nc = bacc.Bacc("TRN2", target_bir_lowering=False, debug=False, num_devices=8)

    hT = nc.dram_tensor("hT", [D, S], MT, kind="ExternalInput")
    w4 = nc.dram_tensor("w4", [D, 2 * FPC], MT, kind="ExternalInput")
    b4 = nc.dram_tensor("b4", [4 * FPC], F32, kind="ExternalInput")
    prot = nc.dram_tensor("prot", [P, P], MT, kind="ExternalInput")
    wvT = nc.dram_tensor("wvT", [D, FPC], MT, kind="ExternalInput")
    bv = nc.dram_tensor("bv", [FPC], MT, kind="ExternalInput")
    cosr = nc.dram_tensor("cosr", [P, S], F32, kind="ExternalInput")
    sinr = nc.dram_tensor("sinr", [P, S], F32, kind="ExternalInput")
    if fast:
        expbT = nc.dram_tensor("expbT", [S, S], F16, kind="ExternalInput")
    else:
        biasT = nc.dram_tensor("biasT", [S, S], F32, kind="ExternalInput")
    owT = nc.dram_tensor("owT", [FPC, D], MT, kind="ExternalInput")
    outT = nc.dram_tensor("outT", [D, S], F32, kind="ExternalOutput")

    hT_r = hT.ap().rearrange("(o p) t -> p o t", p=P)
    w4_r = w4.ap().rearrange("(o p) f -> p o f", p=P)
    wv_r = wvT.ap().rearrange("(o p) f -> p o f", p=P)
    ow_r = owT.ap().rearrange("(o p) f -> p o f", p=P)
    b4_r = b4.ap().rearrange("(o p) -> p o", p=P)

    with tile.TileContext(nc) as tc:
        with (
            tc.tile_pool(name="cst", bufs=1) as cst,
            tc.tile_pool(name="pAB", bufs=1) as pAB,
            tc.tile_pool(name="dram", bufs=4, space="DRAM") as dpool,
        ):
            ident = cst.tile([P, P], F32)
            make_identity(nc, ident)
            prot_sb = cst.tile([P, P], MT)
            nc.sync.dma_start(prot_sb[:], prot.ap())
            ones1 = cst.tile([1, P], MT)
            nc.vector.memset(ones1[:], 1.0)
            b4_sb = cst.tile([P, 4 * FPC // P], F32)
            nc.sync.dma_start(b4_sb[:], b4_r)
            bv_sb = cst.tile([1, FPC], MT)
            eshift = cst.tile([P, 1], F32)
            nc.vector.memset(eshift[:], -12.0)
            nc.sync.dma_start(bv_sb[:], bv.ap()[None, :])

            # persistent phase A->B products
            qk_sb = pAB.tile([P, 2 * FT, S], MT)          # slots: Q ft 0..FT-1, K ft FT..2FT-1
            v_sb = pAB.tile([P, KC, HPC, 66], MT)          # col 64 = ones

            nc.vector.memset(v_sb[:, :, :, 64:65], 1.0)

            PSW = max(NQH, 512)  # psum tag width (fp32 elems per partition)

            # ---------------- Phase A: projections + rope ----------------
            # rotate_half is applied on-device: qpre (pre-RoPE Q/K) is copied
            # PSUM->SBUF fp16 on the (otherwise idle) scalar engine, then one
            # 128-contraction permutation matmul with prot produces the
            # rotated branch -- replacing the second full 1024-contraction
            # projection of the rotated-twin scheme. The perm matmul is
            # emitted one tile late so the PE never waits on the copy.
            with (
                tc.tile_pool(name="pA", bufs=1) as pA,
                tc.tile_pool(name="pAw", bufs=2) as pAw,
                tc.tile_pool(name="psA", bufs=2, space="PSUM") as ppsA,
            ):
                def emit_rot(st):
                    qpre, brcol, sin_ap, tca, dst = st
                    pb = ppsA.tile([P, NTA], F32, tag="pb", name="pb")
                    nc.tensor.matmul(pb[:], prot_sb[:], qpre[:],
                                     start=True, stop=True)
                    tcb = pAw.tile([P, NTA], F32, tag="tcb")
                    nc.vector.scalar_tensor_tensor(
                        tcb[:], pb[:], b4_sb[:, brcol:brcol + 1], sin_ap,
                        op0=mybir.AluOpType.add, op1=mybir.AluOpType.mult)
                    nc.vector.tensor_add(dst, tca[:], tcb[:])

                pend = None
                for half in range(2):
                    tsl = slice(half * NQH, (half + 1) * NQH)
                    h_sb = pA.tile([P, DC, NQH], MT, tag="hT", bufs=2)
                    nc.sync.dma_start(h_sb[:], hT_r[:, :, tsl])
                    cos_sb = pA.tile([P, NQH], F32, tag="cos", bufs=2)
                    nc.sync.dma_start(cos_sb[:], cosr.ap()[:, tsl])
                    sin_sb = pA.tile([P, NQH], F32, tag="sin", bufs=2)
                    nc.sync.dma_start(sin_sb[:], sinr.ap()[:, tsl])

                    # Q/K -> qk_sb
                    for qk in range(2):            # 0 = Q, 1 = K
                        for ft in range(FT):
                            fcol = qk * FPC + ft * P
                            wa = pAw.tile([P, DC, P], MT, tag="wA")
                            nc.sync.dma_start(wa[:], w4_r[:, :, fcol:fcol + P])
                            bcol = (qk * 2 * FPC + ft * P) // P
                            brcol = bcol + FPC // P
                            for tq in range(NQH // NTA):
                                qsl = slice(tq * NTA, (tq + 1) * NTA)
                                pa = ppsA.tile([P, NTA], F32, tag="pa", name="pa")
                                for dc in range(DC):
                                    nc.tensor.matmul(pa[:], wa[:, dc], h_sb[:, dc, qsl],
                                                     start=(dc == 0), stop=(dc == DC - 1))
                                qpre = pAw.tile([P, NTA], MT, tag="qpre")
                                nc.scalar.copy(qpre[:], pa[:])
                                tca = pAw.tile([P, NTA], F32, tag="tca")
                                nc.vector.scalar_tensor_tensor(
                                    tca[:], pa[:], b4_sb[:, bcol:bcol + 1], cos_sb[:, qsl],
                                    op0=mybir.AluOpType.add, op1=mybir.AluOpType.mult)
                                dst = qk_sb[:, qk * FT + ft, half * NQH + tq * NTA:
                                            half * NQH + (tq + 1) * NTA]
                                if pend is not None:
                                    emit_rot(pend)
                                pend = (qpre, brcol, sin_sb[:, qsl], tca, dst)

                    # V for this half: t-tiles within half
                    wvs = pA.tile([P, DC, FPC], MT, tag="wV")
                    nc.sync.dma_start(wvs[:], wv_r)
                    for tt in range(NQH // P):
                        gt = half * (NQH // P) + tt            # global t-tile = k-chunk
                        pv = ppsA.tile([P, FPC], F32, tag="pv", name="pv")
                        for dc in range(DC):
                            nc.tensor.matmul(pv[:], h_sb[:, dc, tt * P:(tt + 1) * P],
                                             wvs[:, dc], start=(dc == 0), stop=False)
                        nc.tensor.matmul(pv[:], ones1[:], bv_sb[:], start=False, stop=True)
                        nc.vector.tensor_copy(v_sb[:, gt, :, 0:64], pv[:])
                emit_rot(pend)

            with tc.tile_pool(name="pBC", bufs=1) as pBC:
                ctxT = pBC.tile([P, FT, S], MT)            # normalized ctx, f-major
                ow_sb = pBC.tile([P, FT, D], MT)
                nc.sync.dma_start(ow_sb[:], ow_r)
                # whole exp(bias) slab resident in SBUF: [k-in-chunk, kc, q]
                eb_slab = pBC.tile([P, KC, S], F16)
                ebr = expbT.ap().rearrange("(kc p) q -> p kc q", p=P)
                for kc4 in range(0, KC, 4):
                    nc.sync.dma_start(eb_slab[:, kc4:kc4 + 4, :],
                                      ebr[:, kc4:kc4 + 4, :])

                # ---------------- Phase B: attention ----------------
                # q chunks of NQ=512; kc processed in pairs packed into one
                # [P, 2*NQ] psS tile so exp covers 2 kc per instruction.
                # psS[hi=0] single-buffered (its exp runs FIRST each pair, so
                # the scores->exp->scores chain is one exp long); psS[hi=1]
                # double-buffered (its exp runs second, off the chain).
                # PSUM banks: s0 2 + s1 4 + ctx 2 = 8.
                NQB = 2 * NQ  # kc-pair-packed width
                # Schraudolph approximate exp: exp(x-12) ~=
                # bitcast_f32(int32(A*x + B)); ~2% rms relative error, used on
                # a subset of kc-pairs to offload the scalar engine.
                SCHRA_A = 12102203.161561485          # 2^23 / ln(2)
                SCHRA_B = float(127 * (1 << 23) - 366393) - 12.0 * SCHRA_A
                OFF_PAIRS = ()
                with (
                    tc.tile_pool(name="pB", bufs=2) as pB,
                    tc.tile_pool(name="psB", bufs=1, space="PSUM") as ppsB,
                ):
                    for hp in range(HPC // 2):
                        for qq in range(S // NQ):
                            qoff = qq * NQ
                            qsl = slice(qoff, qoff + NQ)
                            cps = []
                            for i in range(2):
                                ct = ppsB.tile([P, NQ], F32, tag=f"ctx{i}",
                                               name=f"ctx{i}")
                                cps.append(ct[:65, :])
                            def emit_pv(pp, uu, last=False):
                                for half in range(2):
                                    kc = 2 * pp + half
                                    csl = slice(half * NQ, (half + 1) * NQ)
                                    for hi in range(2):
                                        h = 2 * hp + hi
                                        nc.tensor.matmul(
                                            cps[hi][:],
                                            v_sb[:, kc, h, 0:65],
                                            uu[hi][:, csl],
                                            start=(kc == 0),
                                            stop=(last and half == 1))

                            pending = []
                            for p2 in range(KC // 2):
                                psS = [
                                    ppsB.tile([P, NQB], F32, tag="s0", name="psS0",
                                              bufs=1),
                                    ppsB.tile([P, NQB], F32, tag="s1", name="psS1",
                                              bufs=2),
                                ]
                                # scores: h0/h1 adjacent for row-group overlap
                                prev_mm = None
                                for half in range(2):
                                    kc = 2 * p2 + half
                                    csl = slice(half * NQ, (half + 1) * NQ)
                                    for hi in range(2):
                                        h = 2 * hp + hi
                                        base = 64 * (h % 2)
                                        ft = h // 2
                                        ksl = qk_sb[base:base + 64, FT + ft,
                                                    kc * P:(kc + 1) * P]
                                        qap = qk_sb[base:base + 64, ft, qsl]
                                        mm = nc.tensor.matmul(psS[hi][:, csl], ksl,
                                                              qap, start=True,
                                                              stop=True)
                                        if prev_mm is not None:
                                            add_dep_helper(
                                                mm.ins, prev_mm.ins, sync=False,
                                                reason="scores row-group pairing")
                                        prev_mm = mm
                                off = p2 in OFF_PAIRS
                                u2s = [pB.tile([P, NQB], F16, tag=f"u2{hi}",
                                               bufs=3, name=f"u2{hi}")
                                       for hi in range(2)]
                                if off:
                                    # Schraudolph approx exp on DVE (int32
                                    # bit-trick), bias-multiply on the idle
                                    # gpsimd engine -- relieves the scalar
                                    # engine. Emitted before exp(hi0) so the
                                    # long DVE->gpsimd chain starts early.
                                    ui = pB.tile([P, NQB], I32, tag="ui")
                                    nc.vector.tensor_scalar(
                                        ui[:], psS[1][:], SCHRA_A, SCHRA_B,
                                        op0=mybir.AluOpType.mult,
                                        op1=mybir.AluOpType.add)
                                    nc.gpsimd.tensor_mul(
                                        u2s[1].rearrange("p (a b) -> p a b", a=2),
                                        ui.bitcast(F32).rearrange(
                                            "p (a b) -> p a b", a=2),
                                        eb_slab[:, 2 * p2:2 * p2 + 2, qsl])
                                for hi in range(2):  # hi=0 first: on the chain
                                    if hi == 1 and off:
                                        continue
                                    u_sb = pB.tile([P, NQB], F16, tag=f"u{hi}")
                                    nc.scalar.activation(u_sb[:], psS[hi][:],
                                                         AF.Exp, bias=eshift[:])
                                    nc.vector.tensor_mul(
                                        u2s[hi].rearrange("p (a b) -> p a b", a=2),
                                        u_sb.rearrange("p (a b) -> p a b", a=2),
                                        eb_slab[:, 2 * p2:2 * p2 + 2, qsl])
                                # software-pipeline: PV lags two kc-pairs so
                                # the PE never waits on the exp/mult chain
                                # (the approx path has ~4us of latency).
                                pending.append((p2, u2s))
                                if len(pending) > 2:
                                    pp, uu = pending.pop(0)
                                    emit_pv(pp, uu)
                            while pending:
                                pp, uu = pending.pop(0)
                                emit_pv(pp, uu, last=not pending)
                            # finalize: evacuate ctx PSUM -> SBUF with two
                            # quick scalar copies (frees the ctx banks for the
                            # next chunk's PV immediately); the reciprocal /
                            # broadcast / normalize then run lazily from SBUF
                            # in the next chunk's shadow.
                            cus = []
                            for hi in range(2):
                                cu = pB.tile([65, NQ], F32, tag=f"cu{hi}")
                                if hi == 0:
                                    nc.scalar.copy(cu[:], cps[hi][:])
                                else:
                                    nc.vector.tensor_copy(cu[:], cps[hi][:])
                                cus.append(cu)
                            rscrs, rsqs, rrecs, rscr2s, rbs = [], [], [], [], []
                            for hi in range(2):
                                rscr = dpool.tile([NQ], F32)
                                nc.gpsimd.dma_start(rscr[None, :],
                                                    cus[hi][64:65, :])
                                rscrs.append(rscr)
                            for hi in range(2):
                                rsq = pB.tile([32, NQ // 32], F32, tag=f"rsq{hi}")
                                nc.gpsimd.dma_start(
                                    rsq[:], rscrs[hi].rearrange("(a b) -> a b", a=32))
                                rsqs.append(rsq)
                            for hi in range(2):
                                rrec = pB.tile([32, NQ // 32], F32, tag=f"rrec{hi}")
                                nc.vector.reciprocal(rrec[:], rsqs[hi][:])
                                rrecs.append(rrec)
                            for hi in range(2):
                                rscr2 = dpool.tile([NQ], F32)
                                nc.gpsimd.dma_start(
                                    rscr2.rearrange("(a b) -> a b", a=32), rrecs[hi][:])
                                rscr2s.append(rscr2)
                            for hi in range(2):
                                rb = pB.tile([64, NQ], F32, tag=f"rb{hi}")
                                nc.gpsimd.dma_start(rb[:],
                                                    rscr2s[hi].partition_broadcast(64))
                                rbs.append(rb)
                            for hi in range(2):
                                h = 2 * hp + hi
                                base = 64 * (h % 2)
                                ft = h // 2
                                nc.vector.tensor_mul(
                                    ctxT[base:base + 64, ft, qsl],
                                    cus[hi][0:64, :], rbs[hi][:])

                # ---------------- Phase C: output projection ----------------
                with (
                    tc.tile_pool(name="pC", bufs=4) as pC,
                    tc.tile_pool(name="psC", bufs=4, space="PSUM") as ppsC,
                ):
                    g = 0
                    for tq in range(TT4):
                        for ot in range(D // P):
                            tsl = slice(tq * NT, (tq + 1) * NT)
                            po = ppsC.tile([P, NT], F32, tag="po", name="po")
                            for fc in range(FT):
                                nc.tensor.matmul(po[:],
                                                 ow_sb[:, fc, ot * P:(ot + 1) * P],
                                                 ctxT[:, fc, tsl],
                                                 start=(fc == 0), stop=(fc == FT - 1))
                            o_sb = pC.tile([P, NT], F32, tag="oT")
                            if g % 2 == 0:
                                nc.scalar.copy(o_sb[:], po[:])
                            else:
                                nc.vector.tensor_copy(o_sb[:], po[:])
                            nc.sync.dma_start(outT.ap()[ot * P:(ot + 1) * P, tsl],
                                              o_sb[:])
                            g += 1

    nc.compile()
    return nc


def make_core_inputs(hidden_states, attention_bias, rope_cos, rope_sin, head_mask,
                     qkv_w, qkv_b, o_w, S=2048, D=1024, fast=True):
    """Host-side sharding + layout preparation. Returns list of 8 input dicts."""
    f32 = np.float32
    mt = np.float16 if fast else np.float32
    f16 = np.float16
    hidden_states = np.asarray(hidden_states, f32)
    attention_bias = np.asarray(attention_bias, f32)
    rope_cos = np.asarray(rope_cos, f32)
    rope_sin = np.asarray(rope_sin, f32)
    head_mask = np.asarray(head_mask, f32).reshape(-1)
    qkv_w = np.asarray(qkv_w, f32)
    qkv_b = np.asarray(qkv_b, f32)
    o_w = np.asarray(o_w, f32)

    B = hidden_states.shape[0]
    FPC = HPC * 64
    F = H * 64  # qkv feature dim (row-section size of qkv_w)

    def rot_rows(w):
        # rows indexed by f = hl*64 + d; rot(q)[d] = -q[d+32] (d<32) else q[d-32]
        w = w.reshape(HPC, 64, -1) if w.ndim == 2 else w.reshape(HPC, 64)
        lo, hi = w[:, 0:32], w[:, 32:64]
        out = np.concatenate([-hi, lo], axis=1)
        return out.reshape(HPC * 64, -1) if out.ndim == 3 else out.reshape(HPC * 64)

    cos_t = rope_cos[0, :, 0, :].T.astype(f32)     # [64, S]
    sin_t = rope_sin[0, :, 0, :].T.astype(f32)
    cosr = np.concatenate([cos_t, cos_t], axis=0)  # [128, S]
    sinr = np.concatenate([sin_t, sin_t], axis=0)

    # on-device rotate_half permutation: out[fo] = sum_fi prot[fi, fo] * in[fi]
    protm = np.zeros((128, 128), np.float32)
    for blk in (0, 64):
        for dd in range(32):
            protm[blk + 32 + dd, blk + dd] = -1.0
            protm[blk + dd, blk + 32 + dd] = 1.0

    in_maps = []
    for c in range(8):
        b, g = divmod(c, G)
        fs = slice(g * FPC, (g + 1) * FPC)
        wq = qkv_w[F * 0:F * 1][fs]
        wk = qkv_w[F * 1:F * 2][fs]
        wv = qkv_w[F * 2:F * 3][fs].copy()
        bq = qkv_b[F * 0:F * 1][fs]
        bk = qkv_b[F * 1:F * 2][fs]
        bvv = qkv_b[F * 2:F * 3][fs].copy()
        mask = head_mask[g * HPC:(g + 1) * HPC]
        wv *= np.repeat(mask, 64)[:, None]
        bvv *= np.repeat(mask, 64)
        bqr = rot_rows(bq)
        bkr = rot_rows(bk)
        w4 = np.concatenate([wq.T, wk.T], axis=1)  # [D, 2*FPC]
        b4 = np.concatenate([bq, bqr, bk, bkr])
        bT = np.ascontiguousarray(attention_bias[b, 0].T)
        m = {
            "hT": np.ascontiguousarray(hidden_states[b].T).astype(mt),
            "w4": np.ascontiguousarray(w4).astype(mt),
            "b4": np.ascontiguousarray(b4),
            "wvT": np.ascontiguousarray(wv.T).astype(mt),
            "bv": np.ascontiguousarray(bvv).astype(mt),
            "cosr": np.ascontiguousarray(cosr),
            "sinr": np.ascontiguousarray(sinr),
            "prot": protm.astype(mt),
            "owT": np.ascontiguousarray(o_w[:, g * FPC:(g + 1) * FPC].T).astype(mt),
        }
        if fast:
            m["expbT"] = np.exp(bT).astype(f16)
        else:
            m["biasT"] = bT
        in_maps.append(m)
    return in_maps


def kernel(hidden_states, attention_bias, rope_cos, rope_sin, head_mask,
           qkv_w, qkv_b, o_w, o_b, **_unused):
    from concourse.bass_utils import run_bass_kernel_spmd

    B, S, D = hidden_states.shape
    fast = _CACHE.get("fast", True)
    if "nc" not in _CACHE:
        _CACHE["nc"] = build_nc(S=S, D=D, fast=fast)
    nc = _CACHE["nc"]

    in_maps = make_core_inputs(hidden_states, attention_bias, rope_cos, rope_sin,
                               head_mask, qkv_w, qkv_b, o_w, S=S, D=D, fast=fast)
    res = run_bass_kernel_spmd(nc, in_maps, list(range(8)))
    _CACHE["last_results"] = res

    o_b = np.asarray(o_b, np.float32)
    out = np.empty((B, S, D), np.float32)
    for b in range(B):
        acc = res.results[2 * b]["outT"].T + res.results[2 * b + 1]["outT"].T
        out[b] = acc + o_b[None, :]
    return out



# revision 29
# speedup vs baseline: 1.2253x; 1.0181x over previous
"""Trainium2 Bass kernel for nn_Attention_8143257993917.

Multi-head attention (packed QKV + RoPE + additive bias + softmax + head_mask
+ o_proj), B=4, S=2048, D=1024, H=16 heads, fp32 I/O.

Sharding: 8 cores = 4 batches x 2 head-groups (tensor-parallel over heads).
Core c handles batch b = c // 2 and heads g*8..g*8+8 with g = c % 2.
Each core computes a partial output (its heads' contribution through o_proj);
the host sums the two partials per batch and adds o_b.

Device-side design (per core, fast mode):
- Everything runs in "transposed" feature-major layouts so the big score /
  probability matrices never need an on-chip transpose:
    Q_T, K_T: [f, t] (f = head*64+d on partitions): out[f,t] = wT[d,f].T @ hT.
    RoPE: q' = (q + bq) * cos + (rot(q) + rot(bq)) * sin, where the rotated
      branch comes from a SECOND projection with host-prerotated weights
      (rotate_half is a row permutation+sign of W, so it folds into weights).
    V: [t, f] natural layout, so V chunks [k=128, d=64] are directly the
      stationary operand of the PV matmul. A ones-column appended to V makes
      the PV matmul also produce the softmax denominators (row 64 of ctx).
    scores S_T[k, q] = K_T_chunk.T @ Q_T (contraction d=64), fp16 operands,
      fp32 PSUM accumulate.
    bias: exp(S+b) = exp(S)*exp(b); exp(bias) is precomputed on the host in
      fp16 and applied as one elementwise multiply on VectorE (removes 512
      identity-matmul bias adds from the PE).
    exp on ScalarE (PSUM -> SBUF) with a constant -12 shift (softmax is
      shift-invariant; keeps exp outputs inside fp16 range).
    PV is software-pipelined one k-chunk behind scores/exp/mult so the PE
      never waits on the current chunk's ScalarE/VectorE results.
    softmax denominators: exact VectorE reciprocal on a [32, NQH/32] reshape
      (via a small DRAM round-trip on the gpsimd DMA queues, which also
      broadcasts 1/r across 64 partitions); one TT multiply normalizes ctx
      and moves it PSUM -> SBUF.
    head_mask is folded into the V projection weights/bias on the host.
    o_proj: out_T[o, t] = sum_f o_wT[f, o] * ctx_T[f, t], fp16 operands.
  Matmul dtype is fp16 rather than bf16: same PE throughput, ~8x lower
  quantization error (all value ranges verified to fit fp16 comfortably).
  fp32 matmuls on TRN2 lower to LOW_HIGH double-pass + 2 cycles/column
  streaming (~5x slower than fp16), hence the fp16 datapath with fp32
  accumulation; measured end-to-end relative error vs the fp32 reference
  is ~1.3e-3.
"""

import sys

sys.path.insert(0, "/opt/trn_rl_repo")

import numpy as np

_CACHE = {}

H = 16
HPC = 8  # heads per core
G = 2  # head groups


def build_nc(S=2048, D=1024, fast=True):
    """Build + compile the per-core Bass program (same program on all cores)."""
    import concourse.bass as bass
    from concourse import bacc
    import concourse.mybir as mybir
    import concourse.tile as tile
    from concourse.masks import make_identity
    from concourse.tile_rust import add_dep_helper

    F32 = mybir.dt.float32
    BF16 = mybir.dt.bfloat16
    F16 = mybir.dt.float16
    I32 = mybir.dt.int32
    MT = F16 if fast else F32      # matmul operand dtype
    AF = mybir.ActivationFunctionType

    P = 128
    DC = D // P          # d chunks (contraction for projections)
    KC = S // P          # k chunks (scores contraction)
    NQH = S // 2         # q-half size
    NQ = min(512, NQH)   # matmul free-dim chunk
    NQC = NQH // NQ      # chunks per q-half
    FPC = HPC * 64       # features per core (= 512)
    FT = FPC // P        # f-tiles per tensor (= 4)
    NT = min(512, S)     # phase C t-chunk (matmul out must fit one PSUM bank)
    TT4 = S // NT
    NTA = min(512, NQH)  # phase A t-chunk

    nc = bacc.Bacc("TRN2", target_bir_lowering=False, debug=False, num_devices=8)

    hT = nc.dram_tensor("hT", [D, S], MT, kind="ExternalInput")
    w4 = nc.dram_tensor("w4", [D, 2 * FPC], MT, kind="ExternalInput")
    b4 = nc.dram_tensor("b4", [4 * FPC], F32, kind="ExternalInput")
    prot = nc.dram_tensor("prot", [P, P], MT, kind="ExternalInput")
    wvT = nc.dram_tensor("wvT", [D, FPC], MT, kind="ExternalInput")
    bv = nc.dram_tensor("bv", [FPC], MT, kind="ExternalInput")
    cosr = nc.dram_tensor("cosr", [P, S], F32, kind="ExternalInput")
    sinr = nc.dram_tensor("sinr", [P, S], F32, kind="ExternalInput")
    if fast:
        expbT = nc.dram_tensor("expbT", [S, S], F16, kind="ExternalInput")
    else:
        biasT = nc.dram_tensor("biasT", [S, S], F32, kind="ExternalInput")
    owT = nc.dram_tensor("owT", [FPC, D], MT, kind="ExternalInput")
    outT = nc.dram_tensor("outT", [D, S], F32, kind="ExternalOutput")

    hT_r = hT.ap().rearrange("(o p) t -> p o t", p=P)
    w4_r = w4.ap().rearrange("(o p) f -> p o f", p=P)
    wv_r = wvT.ap().rearrange("(o p) f -> p o f", p=P)
    ow_r = owT.ap().rearrange("(o p) f -> p o f", p=P)
    b4_r = b4.ap().rearrange("(o p) -> p o", p=P)

    with tile.TileContext(nc) as tc:
        with (
            tc.tile_pool(name="cst", bufs=1) as cst,
            tc.tile_pool(name="pAB", bufs=1) as pAB,
            tc.tile_pool(name="dram", bufs=4, space="DRAM") as dpool,
        ):
            ident = cst.tile([P, P], F32)
            make_identity(nc, ident)
            prot_sb = cst.tile([P, P], MT)
            nc.sync.dma_start(prot_sb[:], prot.ap())
            ones1 = cst.tile([1, P], MT)
            nc.vector.memset(ones1[:], 1.0)
            b4_sb = cst.tile([P, 4 * FPC // P], F32)
            nc.sync.dma_start(b4_sb[:], b4_r)
            bv_sb = cst.tile([1, FPC], MT)
            eshift = cst.tile([P, 1], F32)
            nc.vector.memset(eshift[:], -12.0)
            nc.sync.dma_start(bv_sb[:], bv.ap()[None, :])

            # persistent phase A->B products
            qk_sb = pAB.tile([P, 2 * FT, S], MT)          # slots: Q ft 0..FT-1, K ft FT..2FT-1
            v_sb = pAB.tile([P, KC, HPC, 66], MT)          # col 64 = ones

            nc.vector.memset(v_sb[:, :, :, 64:65], 1.0)

            PSW = max(NQH, 512)  # psum tag width (fp32 elems per partition)

            # ---------------- Phase A: projections + rope ----------------
            # rotate_half is applied on-device: qpre (pre-RoPE Q/K) is copied
            # PSUM->SBUF fp16 on the (otherwise idle) scalar engine, then one
            # 128-contraction permutation matmul with prot produces the
            # rotated branch -- replacing the second full 1024-contraction
            # projection of the rotated-twin scheme. The perm matmul is
            # emitted one tile late so the PE never waits on the copy.
            with (
                tc.tile_pool(name="pA", bufs=1) as pA,
                tc.tile_pool(name="pAw", bufs=2) as pAw,
                tc.tile_pool(name="psA", bufs=2, space="PSUM") as ppsA,
            ):
                def emit_rot(st):
                    qpre, brcol, sin_ap, tca, dst = st
                    pb = ppsA.tile([P, NTA], F32, tag="pb", name="pb")
                    nc.tensor.matmul(pb[:], prot_sb[:], qpre[:],
                                     start=True, stop=True)
                    tcb = pAw.tile([P, NTA], F32, tag="tcb")
                    nc.vector.scalar_tensor_tensor(
                        tcb[:], pb[:], b4_sb[:, brcol:brcol + 1], sin_ap,
                        op0=mybir.AluOpType.add, op1=mybir.AluOpType.mult)
                    nc.vector.tensor_add(dst, tca[:], tcb[:])

                pend = None
                for half in range(2):
                    tsl = slice(half * NQH, (half + 1) * NQH)
                    h_sb = pA.tile([P, DC, NQH], MT, tag="hT", bufs=2)
                    nc.sync.dma_start(h_sb[:], hT_r[:, :, tsl])
                    cos_sb = pA.tile([P, NQH], F32, tag="cos", bufs=2)
                    nc.sync.dma_start(cos_sb[:], cosr.ap()[:, tsl])
                    sin_sb = pA.tile([P, NQH], F32, tag="sin", bufs=2)
                    nc.sync.dma_start(sin_sb[:], sinr.ap()[:, tsl])

                    # Q/K -> qk_sb
                    for qk in range(2):            # 0 = Q, 1 = K
                        for ft in range(FT):
                            fcol = qk * FPC + ft * P
                            wa = pAw.tile([P, DC, P], MT, tag="wA")
                            nc.sync.dma_start(wa[:], w4_r[:, :, fcol:fcol + P])
                            bcol = (qk * 2 * FPC + ft * P) // P
                            brcol = bcol + FPC // P
                            for tq in range(NQH // NTA):
                                qsl = slice(tq * NTA, (tq + 1) * NTA)
                                pa = ppsA.tile([P, NTA], F32, tag="pa", name="pa")
                                for dc in range(DC):
                                    nc.tensor.matmul(pa[:], wa[:, dc], h_sb[:, dc, qsl],
                                                     start=(dc == 0), stop=(dc == DC - 1))
                                qpre = pAw.tile([P, NTA], MT, tag="qpre")
                                nc.scalar.copy(qpre[:], pa[:])
                                tca = pAw.tile([P, NTA], F32, tag="tca")
                                nc.vector.scalar_tensor_tensor(
                                    tca[:], pa[:], b4_sb[:, bcol:bcol + 1], cos_sb[:, qsl],
                                    op0=mybir.AluOpType.add, op1=mybir.AluOpType.mult)
                                dst = qk_sb[:, qk * FT + ft, half * NQH + tq * NTA:
                                            half * NQH + (tq + 1) * NTA]
                                if pend is not None:
                                    emit_rot(pend)
                                pend = (qpre, brcol, sin_sb[:, qsl], tca, dst)

                    # V for this half: t-tiles within half
                    wvs = pA.tile([P, DC, FPC], MT, tag="wV")
                    nc.sync.dma_start(wvs[:], wv_r)
                    for tt in range(NQH // P):
                        gt = half * (NQH // P) + tt            # global t-tile = k-chunk
                        pv = ppsA.tile([P, FPC], F32, tag="pv", name="pv")
                        for dc in range(DC):
                            nc.tensor.matmul(pv[:], h_sb[:, dc, tt * P:(tt + 1) * P],
                                             wvs[:, dc], start=(dc == 0), stop=False)
                        nc.tensor.matmul(pv[:], ones1[:], bv_sb[:], start=False, stop=True)
                        nc.scalar.copy(v_sb[:, gt, :, 0:64], pv[:])
                emit_rot(pend)

            with tc.tile_pool(name="pBC", bufs=1) as pBC:
                ctxT = pBC.tile([P, FT, S], MT)            # normalized ctx, f-major
                ow_sb = pBC.tile([P, FT, D], MT)
                nc.sync.dma_start(ow_sb[:], ow_r)
                # whole exp(bias) slab resident in SBUF: [k-in-chunk, kc, q]
                eb_slab = pBC.tile([P, KC, S], F16)
                ebr = expbT.ap().rearrange("(kc p) q -> p kc q", p=P)
                for kc4 in range(0, KC, 4):
                    nc.sync.dma_start(eb_slab[:, kc4:kc4 + 4, :],
                                      ebr[:, kc4:kc4 + 4, :])

                # ---------------- Phase B: attention ----------------
                # q chunks of NQ=512; kc processed in pairs packed into one
                # [P, 2*NQ] psS tile so exp covers 2 kc per instruction.
                # psS[hi=0] single-buffered (its exp runs FIRST each pair, so
                # the scores->exp->scores chain is one exp long); psS[hi=1]
                # double-buffered (its exp runs second, off the chain).
                # PSUM banks: s0 2 + s1 4 + ctx 2 = 8.
                NQB = 2 * NQ  # kc-pair-packed width
                # Schraudolph approximate exp: exp(x-12) ~=
                # bitcast_f32(int32(A*x + B)); ~2% rms relative error, used on
                # a subset of kc-pairs to offload the scalar engine.
                SCHRA_A = 12102203.161561485          # 2^23 / ln(2)
                SCHRA_B = float(127 * (1 << 23) - 366393) - 12.0 * SCHRA_A
                OFF_PAIRS = ()
                with (
                    tc.tile_pool(name="pB", bufs=2) as pB,
                    tc.tile_pool(name="psB", bufs=1, space="PSUM") as ppsB,
                ):
                    for hp in range(HPC // 2):
                        for qq in range(S // NQ):
                            qoff = qq * NQ
                            qsl = slice(qoff, qoff + NQ)
                            cps = []
                            for i in range(2):
                                ct = ppsB.tile([P, NQ], F32, tag=f"ctx{i}",
                                               name=f"ctx{i}")
                                cps.append(ct[:65, :])
                            def emit_pv(pp, uu, last=False):
                                for half in range(2):
                                    kc = 2 * pp + half
                                    csl = slice(half * NQ, (half + 1) * NQ)
                                    for hi in range(2):
                                        h = 2 * hp + hi
                                        nc.tensor.matmul(
                                            cps[hi][:],
                                            v_sb[:, kc, h, 0:65],
                                            uu[hi][:, csl],
                                            start=(kc == 0),
                                            stop=(last and half == 1))

                            pending = []
                            for p2 in range(KC // 2):
                                psS = [
                                    ppsB.tile([P, NQB], F32, tag="s0", name="psS0",
                                              bufs=1),
                                    ppsB.tile([P, NQB], F32, tag="s1", name="psS1",
                                              bufs=2),
                                ]
                                # scores: h0/h1 adjacent for row-group overlap
                                prev_mm = None
                                for half in range(2):
                                    kc = 2 * p2 + half
                                    csl = slice(half * NQ, (half + 1) * NQ)
                                    for hi in range(2):
                                        h = 2 * hp + hi
                                        base = 64 * (h % 2)
                                        ft = h // 2
                                        ksl = qk_sb[base:base + 64, FT + ft,
                                                    kc * P:(kc + 1) * P]
                                        qap = qk_sb[base:base + 64, ft, qsl]
                                        mm = nc.tensor.matmul(psS[hi][:, csl], ksl,
                                                              qap, start=True,
                                                              stop=True)
                                        if prev_mm is not None:
                                            add_dep_helper(
                                                mm.ins, prev_mm.ins, sync=False,
                                                reason="scores row-group pairing")
                                        prev_mm = mm
                                off = p2 in OFF_PAIRS
                                u2s = [pB.tile([P, NQB], F16, tag=f"u2{hi}",
                                               bufs=3, name=f"u2{hi}")
                                       for hi in range(2)]
                                if off:
                                    # Schraudolph approx exp on DVE (int32
                                    # bit-trick), bias-multiply on the idle
                                    # gpsimd engine -- relieves the scalar
                                    # engine. Emitted before exp(hi0) so the
                                    # long DVE->gpsimd chain starts early.
                                    ui = pB.tile([P, NQB], I32, tag="ui")
                                    nc.vector.tensor_scalar(
                                        ui[:], psS[1][:], SCHRA_A, SCHRA_B,
                                        op0=mybir.AluOpType.mult,
                                        op1=mybir.AluOpType.add)
                                    nc.gpsimd.tensor_mul(
                                        u2s[1].rearrange("p (a b) -> p a b", a=2),
                                        ui.bitcast(F32).rearrange(
                                            "p (a b) -> p a b", a=2),
                                        eb_slab[:, 2 * p2:2 * p2 + 2, qsl])
                                for hi in range(2):  # hi=0 first: on the chain
                                    if hi == 1 and off:
                                        continue
                                    u_sb = pB.tile([P, NQB], F16, tag=f"u{hi}")
                                    nc.scalar.activation(u_sb[:], psS[hi][:],
                                                         AF.Exp, bias=eshift[:])
                                    nc.vector.tensor_mul(
                                        u2s[hi].rearrange("p (a b) -> p a b", a=2),
                                        u_sb.rearrange("p (a b) -> p a b", a=2),
                                        eb_slab[:, 2 * p2:2 * p2 + 2, qsl])
                                # software-pipeline: PV lags two kc-pairs so
                                # the PE never waits on the exp/mult chain
                                # (the approx path has ~4us of latency).
                                pending.append((p2, u2s))
                                if len(pending) > 2:
                                    pp, uu = pending.pop(0)
                                    emit_pv(pp, uu)
                            while pending:
                                pp, uu = pending.pop(0)
                                emit_pv(pp, uu, last=not pending)
                            # finalize: evacuate ctx PSUM -> SBUF with two
                            # quick scalar copies (frees the ctx banks for the
                            # next chunk's PV immediately); the reciprocal /
                            # broadcast / normalize then run lazily from SBUF
                            # in the next chunk's shadow.
                            cus = []
                            for hi in range(2):
                                cu = pB.tile([65, NQ], F32, tag=f"cu{hi}")
                                if hi == 0:
                                    nc.scalar.copy(cu[:], cps[hi][:])
                                else:
                                    nc.vector.tensor_copy(cu[:], cps[hi][:])
                                cus.append(cu)
                            rscrs, rsqs, rrecs, rscr2s, rbs = [], [], [], [], []
                            for hi in range(2):
                                rscr = dpool.tile([NQ], F32)
                                nc.gpsimd.dma_start(rscr[None, :],
                                                    cus[hi][64:65, :])
                                rscrs.append(rscr)
                            for hi in range(2):
                                rsq = pB.tile([32, NQ // 32], F32, tag=f"rsq{hi}")
                                nc.gpsimd.dma_start(
                                    rsq[:], rscrs[hi].rearrange("(a b) -> a b", a=32))
                                rsqs.append(rsq)
                            for hi in range(2):
                                rrec = pB.tile([32, NQ // 32], F32, tag=f"rrec{hi}")
                                nc.vector.reciprocal(rrec[:], rsqs[hi][:])
                                rrecs.append(rrec)
                            for hi in range(2):
                                rscr2 = dpool.tile([NQ], F32)
                                nc.gpsimd.dma_start(
                                    rscr2.rearrange("(a b) -> a b", a=32), rrecs[hi][:])
                                rscr2s.append(rscr2)
                            for hi in range(2):
                                rb = pB.tile([64, NQ], F32, tag=f"rb{hi}")
                                nc.gpsimd.dma_start(rb[:],
                                                    rscr2s[hi].partition_broadcast(64))
                                rbs.append(rb)
                            for hi in range(2):
                                h = 2 * hp + hi
                                base = 64 * (h % 2)
                                ft = h // 2
                                nc.vector.tensor_mul(
                                    ctxT[base:base + 64, ft, qsl],
                                    cus[hi][0:64, :], rbs[hi][:])

                # ---------------- Phase C: output projection ----------------
                with (
                    tc.tile_pool(name="pC", bufs=4) as pC,
                    tc.tile_pool(name="psC", bufs=4, space="PSUM") as ppsC,
                ):
                    g = 0
                    for tq in range(TT4):
                        for ot in range(D // P):
                            tsl = slice(tq * NT, (tq + 1) * NT)
                            po = ppsC.tile([P, NT], F32, tag="po", name="po")
                            for fc in range(FT):
                                nc.tensor.matmul(po[:],
                                                 ow_sb[:, fc, ot * P:(ot + 1) * P],
                                                 ctxT[:, fc, tsl],
                                                 start=(fc == 0), stop=(fc == FT - 1))
                            o_sb = pC.tile([P, NT], F32, tag="oT")
                            if g % 2 == 0:
                                nc.scalar.copy(o_sb[:], po[:])
                            else:
                                nc.vector.tensor_copy(o_sb[:], po[:])
                            nc.sync.dma_start(outT.ap()[ot * P:(ot + 1) * P, tsl],
                                              o_sb[:])
                            g += 1

    nc.compile()
    return nc


def make_core_inputs(hidden_states, attention_bias, rope_cos, rope_sin, head_mask,
                     qkv_w, qkv_b, o_w, S=2048, D=1024, fast=True):
    """Host-side sharding + layout preparation. Returns list of 8 input dicts."""
    f32 = np.float32
    mt = np.float16 if fast else np.float32
    f16 = np.float16
    hidden_states = np.asarray(hidden_states, f32)
    attention_bias = np.asarray(attention_bias, f32)
    rope_cos = np.asarray(rope_cos, f32)
    rope_sin = np.asarray(rope_sin, f32)
    head_mask = np.asarray(head_mask, f32).reshape(-1)
    qkv_w = np.asarray(qkv_w, f32)
    qkv_b = np.asarray(qkv_b, f32)
    o_w = np.asarray(o_w, f32)

    B = hidden_states.shape[0]
    FPC = HPC * 64
    F = H * 64  # qkv feature dim (row-section size of qkv_w)

    def rot_rows(w):
        # rows indexed by f = hl*64 + d; rot(q)[d] = -q[d+32] (d<32) else q[d-32]
        w = w.reshape(HPC, 64, -1) if w.ndim == 2 else w.reshape(HPC, 64)
        lo, hi = w[:, 0:32], w[:, 32:64]
        out = np.concatenate([-hi, lo], axis=1)
        return out.reshape(HPC * 64, -1) if out.ndim == 3 else out.reshape(HPC * 64)

    cos_t = rope_cos[0, :, 0, :].T.astype(f32)     # [64, S]
    sin_t = rope_sin[0, :, 0, :].T.astype(f32)
    cosr = np.concatenate([cos_t, cos_t], axis=0)  # [128, S]
    sinr = np.concatenate([sin_t, sin_t], axis=0)

    # on-device rotate_half permutation: out[fo] = sum_fi prot[fi, fo] * in[fi]
    protm = np.zeros((128, 128), np.float32)
    for blk in (0, 64):
        for dd in range(32):
            protm[blk + 32 + dd, blk + dd] = -1.0
            protm[blk + dd, blk + 32 + dd] = 1.0

    in_maps = []
    for c in range(8):
        b, g = divmod(c, G)
        fs = slice(g * FPC, (g + 1) * FPC)
        wq = qkv_w[F * 0:F * 1][fs]
        wk = qkv_w[F * 1:F * 2][fs]
        wv = qkv_w[F * 2:F * 3][fs].copy()
        bq = qkv_b[F * 0:F * 1][fs]
        bk = qkv_b[F * 1:F * 2][fs]
        bvv = qkv_b[F * 2:F * 3][fs].copy()
        mask = head_mask[g * HPC:(g + 1) * HPC]
        wv *= np.repeat(mask, 64)[:, None]
        bvv *= np.repeat(mask, 64)
        bqr = rot_rows(bq)
        bkr = rot_rows(bk)
        w4 = np.concatenate([wq.T, wk.T], axis=1)  # [D, 2*FPC]
        b4 = np.concatenate([bq, bqr, bk, bkr])
        bT = np.ascontiguousarray(attention_bias[b, 0].T)
        m = {
            "hT": np.ascontiguousarray(hidden_states[b].T).astype(mt),
            "w4": np.ascontiguousarray(w4).astype(mt),
            "b4": np.ascontiguousarray(b4),
            "wvT": np.ascontiguousarray(wv.T).astype(mt),
            "bv": np.ascontiguousarray(bvv).astype(mt),
            "cosr": np.ascontiguousarray(cosr),
            "sinr": np.ascontiguousarray(sinr),
            "prot": protm.astype(mt),
            "owT": np.ascontiguousarray(o_w[:, g * FPC:(g + 1) * FPC].T).astype(mt),
        }
        if fast:
            m["expbT"] = np.exp(bT).astype(f16)
        else:
            m["biasT"] = bT
        in_maps.append(m)
    return in_maps


def kernel(hidden_states, attention_bias, rope_cos, rope_sin, head_mask,
           qkv_w, qkv_b, o_w, o_b, **_unused):
    from concourse.bass_utils import run_bass_kernel_spmd

    B, S, D = hidden_states.shape
    fast = _CACHE.get("fast", True)
    if "nc" not in _CACHE:
        _CACHE["nc"] = build_nc(S=S, D=D, fast=fast)
    nc = _CACHE["nc"]

    in_maps = make_core_inputs(hidden_states, attention_bias, rope_cos, rope_sin,
                               head_mask, qkv_w, qkv_b, o_w, S=S, D=D, fast=fast)
    res = run_bass_kernel_spmd(nc, in_maps, list(range(8)))
    _CACHE["last_results"] = res

    o_b = np.asarray(o_b, np.float32)
    out = np.empty((B, S, D), np.float32)
    for b in range(B):
        acc = res.results[2 * b]["outT"].T + res.results[2 * b + 1]["outT"].T
        out[b] = acc + o_b[None, :]
    return out

